# revision 44
# baseline (speedup 1.0000x reference)
"""Distributed Trainium2 Bass kernel for a llama-style GQA attention block.

Problem: x[2,2048,4096] -> QKV proj, interleaved RoPE, causal GQA attention
(32 q heads / 8 kv heads), output proj -> out[2,2048,4096], f32.

Strategy: context parallelism over tokens (NOT the tensor-parallel hint: an
output all-reduce of 67 MB would cost as much as all the compute; here the
only collectives are two ~1 MB/rank bf16 all-gathers of K and V).

  - core c => batch b=c//4, in-batch rank j=c%4. Each core owns 512 query
    tokens of its batch: the two 256-token stripes {j, 7-j} (of 8), so the
    causal work is balanced across cores.
  - each core computes Q/K/V projections for its own 512 tokens, applies
    RoPE, all-gathers K and V (bf16) within its 4-core batch group, runs
    attention for all 32 heads over its queries, applies the full output
    projection to its rows, and writes a disjoint slice of the output.

SPMD: all 8 cores execute ONE compiled program, so the causal structure
cannot appear as per-core control flow. The program processes, per
(head pair, 256-query stripe slot): the union over cores of possibly-valid
gathered key tiles (6 for slot 0, 14 for slot 1), where per-core invalid
tiles are zeroed by a host-provided additive exp bias (data, not code),
plus 2 diagonal key tiles from the core's LOCAL pre-gather K/V (fixed tile
positions on every core) with triangular masks. Both heads of a q-head
pair share their kv head, bias, and masks, so scores/exp/PV/denominator
run once per pair at free dim 512 = [head0 | head1].

Numerics: bf16 matmuls with f32 PSUM accumulation. Softmax without max
subtraction (scores are ~N(0,1); exp cannot overflow). The 1/sqrt(128)
score scale is folded into Q's RoPE tables. The interleaved RoPE pairs are
de-interleaved by permuting wq/wk rows on the host (QK^T is invariant to a
shared intra-head permutation), making RoPE contiguous [64, T] vector ops.
"""

import sys

sys.path.insert(0, "/opt/trn_rl_repo")

import numpy as np
import ml_dtypes

import concourse.bass as bass
import concourse.mybir as mybir
import concourse.tile as tile
from concourse import bacc
from concourse.bass_utils import run_bass_kernel_spmd

# problem dims
DIM = 4096
N_HEADS = 32
N_KV_HEADS = 8
HEAD_DIM = 128
BSZ, SEQLEN = 2, 2048

N_CORES = 8
CPB = 4            # cores per batch
T_LOC = 512        # query tokens per core
STRIPE = 256       # 256-token query stripe; 2 per core
P = 128
DM_TILES = DIM // P  # 32
N_GT = 16          # gathered 128-token key tiles per batch

BF16 = mybir.dt.bfloat16
F32 = mybir.dt.float32
NEG = -30000.0

# abs 128-token tile index of each gathered tile position (one batch group):
# in-batch rank r contributes stripes r then 7-r, each two 128-tiles.
A_MAP = []
for _r in range(CPB):
    A_MAP += [2 * _r, 2 * _r + 1, 2 * (7 - _r), 2 * (7 - _r) + 1]


def build_kernel():
    nc = bacc.Bacc("TRN2", target_bir_lowering=False, debug=False,
                   num_devices=N_CORES)

    # ---- per-core inputs (host-prepped layouts, see _prep_inputs) ----
    xt_ext = nc.declare_dram_parameter("xt", [DM_TILES, P, T_LOC], BF16,
                                       isOutput=False)
    wqt_ext = nc.declare_dram_parameter("wqt", [32, P, DM_TILES, P], BF16,
                                        isOutput=False)
    wkt_ext = nc.declare_dram_parameter("wkt", [N_KV_HEADS, P, DM_TILES, P],
                                        BF16, isOutput=False)
    wvt_ext = nc.declare_dram_parameter("wvt", [2, 4, P, 8, 512], BF16,
                                        isOutput=False)
    wot_ext = nc.declare_dram_parameter("wot", [8, P, DM_TILES, 512], BF16,
                                        isOutput=False)
    cosq_ext = nc.declare_dram_parameter("cosq", [64, T_LOC], BF16, isOutput=False)
    sinq_ext = nc.declare_dram_parameter("sinq", [64, T_LOC], BF16, isOutput=False)
    cosk_ext = nc.declare_dram_parameter("cosk", [64, T_LOC], BF16, isOutput=False)
    sink_ext = nc.declare_dram_parameter("sink", [64, T_LOC], BF16, isOutput=False)
    mask1_ext = nc.declare_dram_parameter("mask1", [P, T_LOC], BF16, isOutput=False)
    mask2_ext = nc.declare_dram_parameter("mask2", [P, T_LOC], BF16, isOutput=False)
    bias_ext = nc.declare_dram_parameter("bias", [P, 2, N_GT], F32, isOutput=False)
    out_ext = nc.declare_dram_parameter("out", [T_LOC, DIM], F32, isOutput=True)

    groups = [[0, 1, 2, 3], [4, 5, 6, 7]]

    with tile.TileContext(nc) as tc:
        with (
            tc.tile_pool(name="res", bufs=1) as res,
            tc.tile_pool(name="qa", bufs=19) as qa,
            tc.tile_pool(name="dram", bufs=1, space="DRAM") as dram,
        ):
            # ---------- resident tiles ----------
            x_sb = res.tile([P, DM_TILES, T_LOC], BF16)
            ksend = res.tile([P, N_KV_HEADS, T_LOC], BF16)
            vsend = res.tile([P, 4, 1024], BF16)
            cosq = res.tile([64, T_LOC], BF16)
            sinq = res.tile([64, T_LOC], BF16)
            cosk = res.tile([64, T_LOC], BF16)
            sink = res.tile([64, T_LOC], BF16)
            maskp1 = res.tile([P, T_LOC], BF16)
            maskp2 = res.tile([P, T_LOC], BF16)
            bias_sb = res.tile([P, 2, N_GT], F32)
            ones_col = res.tile([P, 1], BF16)
            nc.vector.memset(ones_col[:], 1.0)

            for i in range(DM_TILES):
                nc.sync.dma_start(x_sb[:, i], xt_ext[i])
            nc.sync.dma_start(cosq[:], cosq_ext[:])
            nc.sync.dma_start(sinq[:], sinq_ext[:])
            nc.sync.dma_start(cosk[:], cosk_ext[:])
            nc.sync.dma_start(sink[:], sink_ext[:])
            nc.sync.dma_start(maskp1[:], mask1_ext[:])
            nc.sync.dma_start(maskp2[:], mask2_ext[:])
            nc.sync.dma_start(bias_sb[:], bias_ext[:])

            cc_k_in = dram.tile([N_KV_HEADS, P, T_LOC], BF16)
            cc_k_out = dram.tile([CPB, N_KV_HEADS, P, T_LOC], BF16)
            cc_v_in = dram.tile([4, P, 1024], BF16)
            cc_v_out = dram.tile([CPB, 4, P, 1024], BF16)

            # ---------- phase 1: K/V projection + rope K + all-gather ----
            with (
                tc.tile_pool(name="p1w", bufs=7) as p1w,
                tc.tile_pool(name="rt", bufs=2) as rt,
                tc.tile_pool(name="ps1", bufs=2, space="PSUM") as ps1,
            ):
                for g in range(N_KV_HEADS):
                    wk_g = p1w.tile([P, DM_TILES, P], BF16, tag="wk", bufs=2)
                    for d8 in range(0, DM_TILES, 8):
                        nc.gpsimd.dma_start(wk_g[:, d8:d8 + 8],
                                            wkt_ext[g, :, d8:d8 + 8])
                    ps_k = ps1.tile([P, T_LOC], F32, tag="pj")
                    for dm in range(DM_TILES):
                        nc.tensor.matmul(ps_k[:], wk_g[:, dm], x_sb[:, dm],
                                         start=(dm == 0),
                                         stop=(dm == DM_TILES - 1))
                    _rope(nc, rt, ps_k, cosk, sink, ksend[:, g])
                    nc.sync.dma_start(cc_k_in[g], ksend[:, g])

                nc.gpsimd.collective_compute(
                    "AllGather", mybir.AluOpType.bypass, replica_groups=groups,
                    ins=[cc_k_in[:]], outs=[cc_k_out[:]])

                # prefetch the first Q weight blocks so the Q projection can
                # start the moment the V projection drains
                def _wq_fetch(h):
                    blk = p1w.tile([P, DM_TILES, P], BF16, tag="wqblk",
                                   bufs=4, name=f"wqb{h}")
                    for dmf in range(0, DM_TILES, 8):
                        nc.sync.dma_start(blk[:, dmf:dmf + 8],
                                          wqt_ext[h, :, dmf:dmf + 8])
                    return blk

                wq_blks = {hh: _wq_fetch(hh) for hh in range(3)}

                for vh in range(2):
                    ps_v = [
                        ps1.tile([P, 512], F32, tag=f"pvt{tt}", bufs=1,
                                 name=f"psv{vh}{tt}")
                        for tt in range(4)
                    ]
                    for grp in range(4):
                        wv_b = p1w.tile([P, 8, 512], BF16, tag="wv", bufs=2,
                                        name=f"wvb{vh}{grp}")
                        for i2 in range(0, 8, 2):
                            nc.gpsimd.dma_start(
                                wv_b[:, i2:i2 + 2],
                                wvt_ext[vh, grp, :, i2:i2 + 2])
                        for d8 in range(8):
                            dm = grp * 8 + d8
                            for tt in range(4):
                                nc.tensor.matmul(
                                    ps_v[tt][:],
                                    x_sb[:, dm, tt * P:(tt + 1) * P],
                                    wv_b[:, d8], start=(dm == 0),
                                    stop=(dm == DM_TILES - 1))
                    for tt in range(4):
                        nc.scalar.copy(vsend[:, tt, vh * 512:(vh + 1) * 512],
                                       ps_v[tt][:])
                for tt in range(4):
                    nc.sync.dma_start(cc_v_in[tt], vsend[:, tt])

                nc.gpsimd.collective_compute(
                    "AllGather", mybir.AluOpType.bypass, replica_groups=groups,
                    ins=[cc_v_in[:]], outs=[cc_v_out[:]])

                # -------- phase 2: Q projection + rope (overlaps gathers) --
                # Roped Q of a head PAIR is stored interleaved as
                # [128, slot, rel_head, 256] so attention can consume both
                # heads of a pair with single N=512 matmuls.
                qt = []
                for pair in range(16):
                    qp_t = qa.tile([P, 2, 2, STRIPE], BF16, tag="qt",
                                   name=f"qp{pair}")
                    for qh_rel in range(2):
                        h = 2 * pair + qh_rel
                        wq_blk = wq_blks.pop(h) if h in wq_blks \
                            else _wq_fetch(h)
                        ps_q = ps1.tile([P, T_LOC], F32, tag="pj")
                        for dm in range(DM_TILES):
                            nc.tensor.matmul(
                                ps_q[:], wq_blk[:, dm], x_sb[:, dm],
                                start=(dm == 0), stop=(dm == DM_TILES - 1))
                        _rope(nc, rt, ps_q, cosq, sinq,
                              qp_t[:, :, qh_rel, :])
                    qt.append(qp_t)

            # ---------- phase 3: attention ----------
            # Per (head, stripe slot) the program iterates only the gathered
            # tiles that can be causally valid on ANY core for that slot
            # (slot 0: abs tile < 2*3, slot 1: abs tile < 2*7), then the two
            # local diagonal tiles. Per-core validity inside that union is
            # data (exp bias 0 / -30000), keeping the program SPMD-uniform
            # with only ~2 wasted tiles per head.
            slot_gts = [
                [gtt for gtt in range(N_GT) if A_MAP[gtt] < 2 * 3],
                [gtt for gtt in range(N_GT) if A_MAP[gtt] < 2 * 7],
            ]
            with (
                tc.tile_pool(name="kvp", bufs=1) as kvp,
                tc.tile_pool(name="at", bufs=8) as at,
                tc.tile_pool(name="ps_sc", bufs=3, space="PSUM") as ps_sc,
                tc.tile_pool(name="ps_pv", bufs=2, space="PSUM") as ps_pv,
                tc.tile_pool(name="ps_dn", bufs=2, space="PSUM") as ps_dn,
            ):
                # read back gathered K/V
                kfull = kvp.tile([P, N_KV_HEADS, CPB * T_LOC], BF16)
                vfull = kvp.tile([P, N_GT, 1024], BF16)
                for g in range(N_KV_HEADS):
                    for r in range(CPB):
                        nc.sync.dma_start(
                            kfull[:, g, r * T_LOC:(r + 1) * T_LOC],
                            cc_k_out[r, g])
                for r in range(CPB):
                    for tt in range(4):
                        nc.sync.dma_start(vfull[:, 4 * r + tt],
                                          cc_v_out[r, tt])
                # Head PAIRS: both q heads of a pair share the kv head, the
                # exp bias, and the masks, so scores / exp / PV / denominator
                # all run once per pair with free dim 512 = [head0 | head1].
                attn = []
                for hp in range(16):
                    g = hp // 2
                    a_p = qa.tile([P, 2, 2, STRIPE], BF16, tag="qt",
                                  name=f"attnp{hp}")
                    attn.append(a_p)
                    for s in range(2):
                        q_ap = qt[hp][:, s]  # [128, 2, 256] = both heads
                        ps_o = ps_pv.tile([P, T_LOC], F32, tag="pv")
                        ps_d = ps_dn.tile([1, T_LOC], F32, tag="dn")
                        tiles = [("g", gtt) for gtt in slot_gts[s]]
                        tiles += [("d", 0), ("d", 1)]
                        n_acc = len(tiles)
                        for i, (kind, idx) in enumerate(tiles):
                            ps_s = ps_sc.tile([P, T_LOC], F32, tag="sc")
                            e_t = at.tile([P, T_LOC], BF16, tag="exp", bufs=8,
                                          name=f"e{hp}{s}{i}")
                            if kind == "g":
                                nc.tensor.matmul(
                                    ps_s[:],
                                    kfull[:, g, idx * P:(idx + 1) * P],
                                    q_ap, start=True, stop=True)
                                nc.scalar.activation(
                                    e_t[:], ps_s[:],
                                    mybir.ActivationFunctionType.Exp,
                                    bias=bias_sb[:, s, idx:idx + 1])
                                v_ap = vfull[:, idx, g * P:(g + 1) * P]
                            else:
                                lt = 2 * s + idx
                                nc.tensor.matmul(
                                    ps_s[:],
                                    ksend[:, g, lt * P:(lt + 1) * P],
                                    q_ap, start=True, stop=True)
                                msk = maskp1 if idx == 0 else maskp2
                                nc.vector.tensor_tensor(
                                    ps_s[:], ps_s[:], msk[:],
                                    mybir.AluOpType.add)
                                nc.scalar.activation(
                                    e_t[:], ps_s[:],
                                    mybir.ActivationFunctionType.Exp)
                                v_ap = vsend[:, lt, g * P:(g + 1) * P]
                            nc.tensor.matmul(
                                ps_o[:], v_ap, e_t[:],
                                start=(i == 0), stop=(i == n_acc - 1))
                            nc.tensor.matmul(
                                ps_d[:], ones_col[:], e_t[:],
                                start=(i == 0), stop=(i == n_acc - 1))
                        # normalize both heads: attn[:, s] = ps_o / denom
                        dn_sb = at.tile([1, T_LOC], F32, tag="dnsb", bufs=2)
                        nc.scalar.copy(dn_sb[:], ps_d[:])
                        dn_b = at.tile([P, T_LOC], F32, tag="dnb", bufs=2)
                        nc.gpsimd.partition_broadcast(dn_b[:], dn_sb[:])
                        rec_b = at.tile([P, T_LOC], F32, tag="recb", bufs=2)
                        nc.vector.reciprocal_approx_fast(rec_b[:], dn_b[:])
                        nc.vector.tensor_tensor(
                            a_p[:, s], ps_o[:], rec_b[:],
                            mybir.AluOpType.mult)

            # ---------- phase 4: output projection ----------
            with (
                tc.tile_pool(name="p5w", bufs=6) as p5w,
                tc.tile_pool(name="p5s", bufs=4) as p5s,
                tc.tile_pool(name="ps5", bufs=1, space="PSUM") as ps5,
            ):
                for ot in range(8):
                    ps_os = [
                        ps5.tile([P, 512], F32, tag=f"po{t4}",
                                 name=f"pso{ot}{t4}")
                        for t4 in range(4)
                    ]
                    for afb in range(8):
                        wo_c = p5w.tile([P, 4, 512], BF16, tag="woc")
                        for a2 in range(0, 4, 2):
                            nc.sync.dma_start(
                                wo_c[:, a2:a2 + 2],
                                wot_ext[ot, :,
                                        afb * 4 + a2:afb * 4 + a2 + 2])
                        for af_rel in range(4):
                            af = afb * 4 + af_rel
                            for t4 in range(4):
                                nc.tensor.matmul(
                                    ps_os[t4][:],
                                    attn[af // 2][:, t4 // 2, af % 2,
                                                  (t4 % 2) * P:
                                                  (t4 % 2 + 1) * P],
                                    wo_c[:, af_rel],
                                    start=(af == 0),
                                    stop=(af == DM_TILES - 1))
                    for t4 in range(4):
                        o_st = p5s.tile([P, 512], F32, tag="ostage",
                                        name=f"ost{ot}{t4}")
                        nc.scalar.copy(o_st[:], ps_os[t4][:])
                        nc.sync.dma_start(
                            out_ext[t4 * P:(t4 + 1) * P,
                                    ot * 512:(ot + 1) * 512], o_st[:])

    nc.finalize()
    return nc


def _rope(nc, pool, ps, cos, sin, out_sb):
    """RoPE on de-interleaved layout.

    ps: [128, T] f32 psum; partitions 0:64 = even dims (a), 64:128 = odd (b).
    out[0:64] = a*cos - b*sin; out[64:128] = a*sin + b*cos.
    """
    T = ps.shape[-1]
    a = ps[0:64]
    b = ps[64:128]
    t0 = pool.tile([64, T], F32, tag="ropet0")
    t1 = pool.tile([64, T], F32, tag="ropet1", bufs=1)
    nc.vector.tensor_tensor(t0[:], a, cos[:], mybir.AluOpType.mult)
    nc.vector.tensor_tensor(t1[:], b, sin[:], mybir.AluOpType.mult)
    nc.vector.tensor_tensor(out_sb[0:64], t0[:], t1[:],
                            mybir.AluOpType.subtract)
    nc.vector.tensor_tensor(t0[:], a, sin[:], mybir.AluOpType.mult)
    nc.vector.tensor_tensor(t1[:], b, cos[:], mybir.AluOpType.mult)
    nc.vector.tensor_tensor(out_sb[64:128], t0[:], t1[:], mybir.AluOpType.add)


# ---------------------------------------------------------------------------
# host side
# ---------------------------------------------------------------------------

def _deint_perm(n_heads):
    """Row permutation de-interleaving rope pairs within each head."""
    idx = []
    for h in range(n_heads):
        base = h * HEAD_DIM
        idx += [base + d for d in range(0, HEAD_DIM, 2)]
        idx += [base + d for d in range(1, HEAD_DIM, 2)]
    return np.array(idx)


def _tokens_of_core(c):
    j = c % CPB
    s1, s2 = j, 7 - j
    return np.concatenate([
        np.arange(s1 * STRIPE, (s1 + 1) * STRIPE),
        np.arange(s2 * STRIPE, (s2 + 1) * STRIPE)])


def _prep_inputs(x, wq, wk, wv, wo, freqs_cos, freqs_sin):
    bf16 = ml_dtypes.bfloat16
    f32 = np.float32

    wq_p = wq[_deint_perm(N_HEADS)]
    wk_p = wk[_deint_perm(N_KV_HEADS)]

    # shared blocked weights
    wqt = np.ascontiguousarray(
        wq_p.T.reshape(DM_TILES, P, N_HEADS, P).transpose(2, 1, 0, 3)
    ).astype(bf16)
    wkt = np.ascontiguousarray(
        wk_p.T.reshape(DM_TILES, P, N_KV_HEADS, P).transpose(2, 1, 0, 3)
    ).astype(bf16)
    wvt = np.ascontiguousarray(
        wv.T.reshape(4, 8, P, 2, 512).transpose(3, 0, 2, 1, 4)).astype(bf16)
    wot = np.ascontiguousarray(
        wo.T.reshape(DM_TILES, P, 8, 512).transpose(2, 1, 0, 3)).astype(bf16)

    inv = np.float32(1.0 / np.sqrt(HEAD_DIM))
    cosT = freqs_cos.T.astype(f32)  # [64, S]
    sinT = freqs_sin.T.astype(f32)
    bf = bf16

    t_idx = np.arange(P)[:, None]
    q_idx = np.arange(STRIPE)[None, :]
    mask1 = np.where(t_idx <= q_idx, 0.0, NEG).astype(f32)
    mask2 = np.where(t_idx + P <= q_idx, 0.0, NEG).astype(f32)
    mask1 = np.concatenate([mask1, mask1], axis=1).astype(bf16)
    mask2 = np.concatenate([mask2, mask2], axis=1).astype(bf16)

    in_maps = []
    for c in range(N_CORES):
        b, j = c // CPB, c % CPB
        tok = _tokens_of_core(c)
        xt = np.ascontiguousarray(
            x[b][tok].T.reshape(DM_TILES, P, T_LOC)).astype(bf16)
        bias = np.zeros((P, 2, N_GT), f32)
        for s in range(2):
            s_abs = j if s == 0 else 7 - j
            for gt in range(N_GT):
                if A_MAP[gt] >= 2 * s_abs:
                    bias[:, s, gt] = NEG
        in_maps.append({
            "xt": xt,
            "wqt": wqt, "wkt": wkt, "wvt": wvt, "wot": wot,
            "cosq": np.ascontiguousarray(cosT[:, tok] * inv).astype(bf16),
            "sinq": np.ascontiguousarray(sinT[:, tok] * inv).astype(bf16),
            "cosk": np.ascontiguousarray(cosT[:, tok]).astype(bf16),
            "sink": np.ascontiguousarray(sinT[:, tok]).astype(bf16),
            "mask1": mask1, "mask2": mask2,
            "bias": bias,
        })
    return in_maps


_NC_CACHE = None


def _get_nc():
    global _NC_CACHE
    if _NC_CACHE is None:
        _NC_CACHE = build_kernel()
    return _NC_CACHE


def kernel(x, wq, wk, wv, wo, freqs_cos, freqs_sin, _trace=False):
    x = np.asarray(x, dtype=np.float32)
    in_maps = _prep_inputs(
        x, np.asarray(wq, np.float32), np.asarray(wk, np.float32),
        np.asarray(wv, np.float32), np.asarray(wo, np.float32),
        np.asarray(freqs_cos, np.float32), np.asarray(freqs_sin, np.float32))
    nc = _get_nc()
    res = run_bass_kernel_spmd(nc, in_maps, core_ids=list(range(N_CORES)),
                               trace=_trace)
    out = np.empty((BSZ, SEQLEN, DIM), np.float32)
    for c in range(N_CORES):
        out[c // CPB, _tokens_of_core(c)] = res.results[c]["out"]
    if _trace:
        kernel.last_exec_time_ns = res.exec_time_ns
        kernel.last_results = res
    return out


if __name__ == "__main__":
    build_kernel()
    print("built ok")


# revision 45
# speedup vs baseline: 1.0389x; 1.0389x over previous
"""Distributed Trainium2 Bass kernel for a llama-style GQA attention block.

Problem: x[2,2048,4096] -> QKV proj, interleaved RoPE, causal GQA attention
(32 q heads / 8 kv heads), output proj -> out[2,2048,4096], f32.

Strategy: context parallelism over tokens (NOT the tensor-parallel hint: an
output all-reduce of 67 MB would cost as much as all the compute; here the
only collectives are two ~1 MB/rank bf16 all-gathers of K and V).

  - core c => batch b=c//4, in-batch rank j=c%4. Each core owns 512 query
    tokens of its batch: the two 256-token stripes {j, 7-j} (of 8), so the
    causal work is balanced across cores.
  - each core computes Q/K/V projections for its own 512 tokens, applies
    RoPE, all-gathers K and V (bf16) within its 4-core batch group, runs
    attention for all 32 heads over its queries, applies the full output
    projection to its rows, and writes a disjoint slice of the output.

SPMD: all 8 cores execute ONE compiled program, so the causal structure
cannot appear as per-core control flow. The program processes, per
(head pair, 256-query stripe slot): the union over cores of possibly-valid
gathered key tiles (6 for slot 0, 14 for slot 1), where per-core invalid
tiles are zeroed by a host-provided additive exp bias (data, not code),
plus 2 diagonal key tiles from the core's LOCAL pre-gather K/V (fixed tile
positions on every core) with triangular masks. Both heads of a q-head
pair share their kv head, bias, and masks, so scores/exp/PV/denominator
run once per pair at free dim 512 = [head0 | head1].

Numerics: bf16 matmuls with f32 PSUM accumulation. Softmax without max
subtraction (scores are ~N(0,1); exp cannot overflow). The 1/sqrt(128)
score scale is folded into Q's RoPE tables. The interleaved RoPE pairs are
de-interleaved by permuting wq/wk rows on the host (QK^T is invariant to a
shared intra-head permutation), making RoPE contiguous [64, T] vector ops.
"""

import sys

sys.path.insert(0, "/opt/trn_rl_repo")

import numpy as np
import ml_dtypes

import concourse.bass as bass
import concourse.mybir as mybir
import concourse.tile as tile
from concourse import bacc
from concourse.bass_utils import run_bass_kernel_spmd

# problem dims
DIM = 4096
N_HEADS = 32
N_KV_HEADS = 8
HEAD_DIM = 128
BSZ, SEQLEN = 2, 2048

N_CORES = 8
CPB = 4            # cores per batch
T_LOC = 512        # query tokens per core
STRIPE = 256       # 256-token query stripe; 2 per core
P = 128
DM_TILES = DIM // P  # 32
N_GT = 16          # gathered 128-token key tiles per batch

BF16 = mybir.dt.bfloat16
F32 = mybir.dt.float32
NEG = -30000.0

# abs 128-token tile index of each gathered tile position (one batch group):
# in-batch rank r contributes stripes r then 7-r, each two 128-tiles.
A_MAP = []
for _r in range(CPB):
    A_MAP += [2 * _r, 2 * _r + 1, 2 * (7 - _r), 2 * (7 - _r) + 1]


def build_kernel():
    nc = bacc.Bacc("TRN2", target_bir_lowering=False, debug=False,
                   num_devices=N_CORES)

    # ---- per-core inputs (host-prepped layouts, see _prep_inputs) ----
    xt_ext = nc.declare_dram_parameter("xt", [DM_TILES, P, T_LOC], BF16,
                                       isOutput=False)
    wqt_ext = nc.declare_dram_parameter("wqt", [32, P, DM_TILES, P], BF16,
                                        isOutput=False)
    wkt_ext = nc.declare_dram_parameter("wkt", [N_KV_HEADS, P, DM_TILES, P],
                                        BF16, isOutput=False)
    wvt_ext = nc.declare_dram_parameter("wvt", [2, 4, P, 8, 512], BF16,
                                        isOutput=False)
    wot_ext = nc.declare_dram_parameter("wot", [8, P, DM_TILES, 512], BF16,
                                        isOutput=False)
    cosq_ext = nc.declare_dram_parameter("cosq", [64, T_LOC], BF16, isOutput=False)
    sinq_ext = nc.declare_dram_parameter("sinq", [64, T_LOC], BF16, isOutput=False)
    cosk_ext = nc.declare_dram_parameter("cosk", [64, T_LOC], BF16, isOutput=False)
    sink_ext = nc.declare_dram_parameter("sink", [64, T_LOC], BF16, isOutput=False)
    mask1_ext = nc.declare_dram_parameter("mask1", [P, T_LOC], BF16, isOutput=False)
    mask2_ext = nc.declare_dram_parameter("mask2", [P, T_LOC], BF16, isOutput=False)
    bias_ext = nc.declare_dram_parameter("bias", [P, 2, N_GT], F32, isOutput=False)
    out_ext = nc.declare_dram_parameter("out", [T_LOC, DIM], F32, isOutput=True)

    groups = [[0, 1, 2, 3], [4, 5, 6, 7]]

    with tile.TileContext(nc) as tc:
        with (
            tc.tile_pool(name="res", bufs=1) as res,
            tc.tile_pool(name="qa", bufs=19) as qa,
            tc.tile_pool(name="dram", bufs=1, space="DRAM") as dram,
        ):
            # ---------- resident tiles ----------
            x_sb = res.tile([P, DM_TILES, T_LOC], BF16)
            ksend = res.tile([P, N_KV_HEADS, T_LOC], BF16)
            vsend = res.tile([P, 4, 1024], BF16)
            cosq = res.tile([64, T_LOC], BF16)
            sinq = res.tile([64, T_LOC], BF16)
            cosk = res.tile([64, T_LOC], BF16)
            sink = res.tile([64, T_LOC], BF16)
            maskp1 = res.tile([P, T_LOC], BF16)
            maskp2 = res.tile([P, T_LOC], BF16)
            bias_sb = res.tile([P, 2, N_GT], F32)
            ones_col = res.tile([P, 1], BF16)
            nc.vector.memset(ones_col[:], 1.0)

            for i in range(DM_TILES):
                nc.sync.dma_start(x_sb[:, i], xt_ext[i])
            nc.sync.dma_start(cosq[:], cosq_ext[:])
            nc.sync.dma_start(sinq[:], sinq_ext[:])
            nc.sync.dma_start(cosk[:], cosk_ext[:])
            nc.sync.dma_start(sink[:], sink_ext[:])
            nc.sync.dma_start(maskp1[:], mask1_ext[:])
            nc.sync.dma_start(maskp2[:], mask2_ext[:])
            nc.sync.dma_start(bias_sb[:], bias_ext[:])

            cc_k_in = dram.tile([N_KV_HEADS, P, T_LOC], BF16)
            cc_k_out = dram.tile([CPB, N_KV_HEADS, P, T_LOC], BF16)
            cc_v_in = dram.tile([4, P, 1024], BF16)
            cc_v_out = dram.tile([CPB, 4, P, 1024], BF16)

            # ---------- phase 1: K/V projection + rope K + all-gather ----
            with (
                tc.tile_pool(name="p1w", bufs=7) as p1w,
                tc.tile_pool(name="rt", bufs=2) as rt,
                tc.tile_pool(name="ps1", bufs=2, space="PSUM") as ps1,
            ):
                for g in range(N_KV_HEADS):
                    wk_g = p1w.tile([P, DM_TILES, P], BF16, tag="wk", bufs=2)
                    for d8 in range(0, DM_TILES, 8):
                        nc.gpsimd.dma_start(wk_g[:, d8:d8 + 8],
                                            wkt_ext[g, :, d8:d8 + 8])
                    ps_k = ps1.tile([P, T_LOC], F32, tag="pj")
                    for dm in range(DM_TILES):
                        nc.tensor.matmul(ps_k[:], wk_g[:, dm], x_sb[:, dm],
                                         start=(dm == 0),
                                         stop=(dm == DM_TILES - 1))
                    _rope(nc, rt, ps_k, cosk, sink, ksend[:, g])
                    nc.sync.dma_start(cc_k_in[g], ksend[:, g])

                nc.gpsimd.collective_compute(
                    "AllGather", mybir.AluOpType.bypass, replica_groups=groups,
                    ins=[cc_k_in[:]], outs=[cc_k_out[:]])

                # prefetch the first Q weight blocks so the Q projection can
                # start the moment the V projection drains
                def _wq_fetch(h):
                    blk = p1w.tile([P, DM_TILES, P], BF16, tag="wqblk",
                                   bufs=4, name=f"wqb{h}")
                    for dmf in range(0, DM_TILES, 8):
                        nc.sync.dma_start(blk[:, dmf:dmf + 8],
                                          wqt_ext[h, :, dmf:dmf + 8])
                    return blk

                wq_blks = {hh: _wq_fetch(hh) for hh in range(3)}

                for vh in range(2):
                    ps_v = [
                        ps1.tile([P, 512], F32, tag=f"pvt{tt}", bufs=1,
                                 name=f"psv{vh}{tt}")
                        for tt in range(4)
                    ]
                    for grp in range(4):
                        wv_b = p1w.tile([P, 8, 512], BF16, tag="wv", bufs=2,
                                        name=f"wvb{vh}{grp}")
                        for i2 in range(0, 8, 2):
                            nc.gpsimd.dma_start(
                                wv_b[:, i2:i2 + 2],
                                wvt_ext[vh, grp, :, i2:i2 + 2])
                        for d8 in range(8):
                            dm = grp * 8 + d8
                            for tt in range(4):
                                nc.tensor.matmul(
                                    ps_v[tt][:],
                                    x_sb[:, dm, tt * P:(tt + 1) * P],
                                    wv_b[:, d8], start=(dm == 0),
                                    stop=(dm == DM_TILES - 1))
                    for tt in range(4):
                        nc.scalar.copy(vsend[:, tt, vh * 512:(vh + 1) * 512],
                                       ps_v[tt][:])
                for tt in range(4):
                    nc.sync.dma_start(cc_v_in[tt], vsend[:, tt])

                nc.gpsimd.collective_compute(
                    "AllGather", mybir.AluOpType.bypass, replica_groups=groups,
                    ins=[cc_v_in[:]], outs=[cc_v_out[:]])

                # -------- phase 2: Q projection + rope (overlaps gathers) --
                # Roped Q of a head PAIR is stored interleaved as
                # [128, slot, rel_head, 256] so attention can consume both
                # heads of a pair with single N=512 matmuls.
                qt = []
                for pair in range(16):
                    qp_t = qa.tile([P, 2, 2, STRIPE], BF16, tag="qt",
                                   name=f"qp{pair}")
                    for qh_rel in range(2):
                        h = 2 * pair + qh_rel
                        wq_blk = wq_blks.pop(h) if h in wq_blks \
                            else _wq_fetch(h)
                        ps_q = ps1.tile([P, T_LOC], F32, tag="pj")
                        for dm in range(DM_TILES):
                            nc.tensor.matmul(
                                ps_q[:], wq_blk[:, dm], x_sb[:, dm],
                                start=(dm == 0), stop=(dm == DM_TILES - 1))
                        _rope(nc, rt, ps_q, cosq, sinq,
                              qp_t[:, :, qh_rel, :])
                    qt.append(qp_t)

            # ---------- phase 3: attention ----------
            # Per (head, stripe slot) the program iterates only the gathered
            # tiles that can be causally valid on ANY core for that slot
            # (slot 0: abs tile < 2*3, slot 1: abs tile < 2*7), then the two
            # local diagonal tiles. Per-core validity inside that union is
            # data (exp bias 0 / -30000), keeping the program SPMD-uniform
            # with only ~2 wasted tiles per head.
            slot_gts = [
                [gtt for gtt in range(N_GT) if A_MAP[gtt] < 2 * 3],
                [gtt for gtt in range(N_GT) if A_MAP[gtt] < 2 * 7],
            ]
            with (
                tc.tile_pool(name="kvp", bufs=1) as kvp,
                tc.tile_pool(name="at", bufs=8) as at,
                tc.tile_pool(name="ps_sc", bufs=3, space="PSUM") as ps_sc,
                tc.tile_pool(name="ps_pv", bufs=2, space="PSUM") as ps_pv,
                tc.tile_pool(name="ps_dn", bufs=2, space="PSUM") as ps_dn,
            ):
                # read back gathered K/V
                kfull = kvp.tile([P, N_KV_HEADS, CPB * T_LOC], BF16)
                vfull = kvp.tile([P, N_GT, 1024], BF16)
                for g in range(N_KV_HEADS):
                    for r in range(CPB):
                        nc.sync.dma_start(
                            kfull[:, g, r * T_LOC:(r + 1) * T_LOC],
                            cc_k_out[r, g])
                for r in range(CPB):
                    for tt in range(4):
                        nc.sync.dma_start(vfull[:, 4 * r + tt],
                                          cc_v_out[r, tt])
                # Head PAIRS: both q heads of a pair share the kv head, the
                # exp bias, and the masks, so scores / exp / PV / denominator
                # all run once per pair with free dim 512 = [head0 | head1].
                attn = []
                for hp in range(16):
                    g = hp // 2
                    a_p = qa.tile([P, 2, 2, STRIPE], BF16, tag="qt",
                                  name=f"attnp{hp}")
                    attn.append(a_p)
                    for s in range(2):
                        q_ap = qt[hp][:, s]  # [128, 2, 256] = both heads
                        ps_o = ps_pv.tile([P, T_LOC], F32, tag="pv")
                        ps_d = ps_dn.tile([1, T_LOC], F32, tag="dn")
                        tiles = [("g", gtt) for gtt in slot_gts[s]]
                        tiles += [("d", 0), ("d", 1)]
                        n_acc = len(tiles)
                        for i, (kind, idx) in enumerate(tiles):
                            ps_s = ps_sc.tile([P, T_LOC], F32, tag="sc")
                            e_t = at.tile([P, T_LOC], BF16, tag="exp", bufs=8,
                                          name=f"e{hp}{s}{i}")
                            if kind == "g":
                                nc.tensor.matmul(
                                    ps_s[:],
                                    kfull[:, g, idx * P:(idx + 1) * P],
                                    q_ap, start=True, stop=True)
                                nc.scalar.activation(
                                    e_t[:], ps_s[:],
                                    mybir.ActivationFunctionType.Exp,
                                    bias=bias_sb[:, s, idx:idx + 1])
                                v_ap = vfull[:, idx, g * P:(g + 1) * P]
                            else:
                                lt = 2 * s + idx
                                nc.tensor.matmul(
                                    ps_s[:],
                                    ksend[:, g, lt * P:(lt + 1) * P],
                                    q_ap, start=True, stop=True)
                                msk = maskp1 if idx == 0 else maskp2
                                nc.vector.tensor_tensor(
                                    ps_s[:], ps_s[:], msk[:],
                                    mybir.AluOpType.add)
                                nc.scalar.activation(
                                    e_t[:], ps_s[:],
                                    mybir.ActivationFunctionType.Exp)
                                v_ap = vsend[:, lt, g * P:(g + 1) * P]
                            nc.tensor.matmul(
                                ps_o[:], v_ap, e_t[:],
                                start=(i == 0), stop=(i == n_acc - 1))
                            nc.tensor.matmul(
                                ps_d[:], ones_col[:], e_t[:],
                                start=(i == 0), stop=(i == n_acc - 1))
                        # normalize both heads: attn[:, s] = ps_o / denom
                        dn_sb = at.tile([1, T_LOC], F32, tag="dnsb", bufs=2)
                        nc.scalar.copy(dn_sb[:], ps_d[:])
                        dn_b = at.tile([P, T_LOC], F32, tag="dnb", bufs=2)
                        nc.gpsimd.partition_broadcast(dn_b[:], dn_sb[:])
                        rec_b = at.tile([P, T_LOC], F32, tag="recb", bufs=2)
                        nc.vector.reciprocal_approx_fast(rec_b[:], dn_b[:])
                        nc.vector.tensor_tensor(
                            a_p[:, s], ps_o[:], rec_b[:],
                            mybir.AluOpType.mult)

            # ---------- phase 4: output projection ----------
            with (
                tc.tile_pool(name="p5w", bufs=8) as p5w,
                tc.tile_pool(name="p5s", bufs=4) as p5s,
                tc.tile_pool(name="ps5", bufs=1, space="PSUM") as ps5,
            ):
                for ot in range(8):
                    ps_os = [
                        ps5.tile([P, 512], F32, tag=f"po{t4}", bufs=2,
                                 name=f"pso{ot}{t4}")
                        for t4 in range(4)
                    ]
                    for afb in range(8):
                        wo_c = p5w.tile([P, 4, 512], BF16, tag="woc")
                        for a2 in range(0, 4, 2):
                            nc.sync.dma_start(
                                wo_c[:, a2:a2 + 2],
                                wot_ext[ot, :,
                                        afb * 4 + a2:afb * 4 + a2 + 2])
                        for af_rel in range(4):
                            af = afb * 4 + af_rel
                            for t4 in range(4):
                                nc.tensor.matmul(
                                    ps_os[t4][:],
                                    attn[af // 2][:, t4 // 2, af % 2,
                                                  (t4 % 2) * P:
                                                  (t4 % 2 + 1) * P],
                                    wo_c[:, af_rel],
                                    start=(af == 0),
                                    stop=(af == DM_TILES - 1))
                    for t4 in range(4):
                        o_st = p5s.tile([P, 512], F32, tag="ostage",
                                        name=f"ost{ot}{t4}")
                        nc.scalar.copy(o_st[:], ps_os[t4][:])
                        nc.sync.dma_start(
                            out_ext[t4 * P:(t4 + 1) * P,
                                    ot * 512:(ot + 1) * 512], o_st[:])

    nc.finalize()
    return nc


def _rope(nc, pool, ps, cos, sin, out_sb):
    """RoPE on de-interleaved layout.

    ps: [128, T] f32 psum; partitions 0:64 = even dims (a), 64:128 = odd (b).
    out[0:64] = a*cos - b*sin; out[64:128] = a*sin + b*cos.
    """
    T = ps.shape[-1]
    a = ps[0:64]
    b = ps[64:128]
    t0 = pool.tile([64, T], F32, tag="ropet0")
    t1 = pool.tile([64, T], F32, tag="ropet1", bufs=1)
    nc.vector.tensor_tensor(t0[:], a, cos[:], mybir.AluOpType.mult)
    nc.vector.tensor_tensor(t1[:], b, sin[:], mybir.AluOpType.mult)
    nc.vector.tensor_tensor(out_sb[0:64], t0[:], t1[:],
                            mybir.AluOpType.subtract)
    nc.vector.tensor_tensor(t0[:], a, sin[:], mybir.AluOpType.mult)
    nc.vector.tensor_tensor(t1[:], b, cos[:], mybir.AluOpType.mult)
    nc.vector.tensor_tensor(out_sb[64:128], t0[:], t1[:], mybir.AluOpType.add)


# ---------------------------------------------------------------------------
# host side
# ---------------------------------------------------------------------------

def _deint_perm(n_heads):
    """Row permutation de-interleaving rope pairs within each head."""
    idx = []
    for h in range(n_heads):
        base = h * HEAD_DIM
        idx += [base + d for d in range(0, HEAD_DIM, 2)]
        idx += [base + d for d in range(1, HEAD_DIM, 2)]
    return np.array(idx)


def _tokens_of_core(c):
    j = c % CPB
    s1, s2 = j, 7 - j
    return np.concatenate([
        np.arange(s1 * STRIPE, (s1 + 1) * STRIPE),
        np.arange(s2 * STRIPE, (s2 + 1) * STRIPE)])


def _prep_inputs(x, wq, wk, wv, wo, freqs_cos, freqs_sin):
    bf16 = ml_dtypes.bfloat16
    f32 = np.float32

    wq_p = wq[_deint_perm(N_HEADS)]
    wk_p = wk[_deint_perm(N_KV_HEADS)]

    # shared blocked weights
    wqt = np.ascontiguousarray(
        wq_p.T.reshape(DM_TILES, P, N_HEADS, P).transpose(2, 1, 0, 3)
    ).astype(bf16)
    wkt = np.ascontiguousarray(
        wk_p.T.reshape(DM_TILES, P, N_KV_HEADS, P).transpose(2, 1, 0, 3)
    ).astype(bf16)
    wvt = np.ascontiguousarray(
        wv.T.reshape(4, 8, P, 2, 512).transpose(3, 0, 2, 1, 4)).astype(bf16)
    wot = np.ascontiguousarray(
        wo.T.reshape(DM_TILES, P, 8, 512).transpose(2, 1, 0, 3)).astype(bf16)

    inv = np.float32(1.0 / np.sqrt(HEAD_DIM))
    cosT = freqs_cos.T.astype(f32)  # [64, S]
    sinT = freqs_sin.T.astype(f32)
    bf = bf16

    t_idx = np.arange(P)[:, None]
    q_idx = np.arange(STRIPE)[None, :]
    mask1 = np.where(t_idx <= q_idx, 0.0, NEG).astype(f32)
    mask2 = np.where(t_idx + P <= q_idx, 0.0, NEG).astype(f32)
    mask1 = np.concatenate([mask1, mask1], axis=1).astype(bf16)
    mask2 = np.concatenate([mask2, mask2], axis=1).astype(bf16)

    in_maps = []
    for c in range(N_CORES):
        b, j = c // CPB, c % CPB
        tok = _tokens_of_core(c)
        xt = np.ascontiguousarray(
            x[b][tok].T.reshape(DM_TILES, P, T_LOC)).astype(bf16)
        bias = np.zeros((P, 2, N_GT), f32)
        for s in range(2):
            s_abs = j if s == 0 else 7 - j
            for gt in range(N_GT):
                if A_MAP[gt] >= 2 * s_abs:
                    bias[:, s, gt] = NEG
        in_maps.append({
            "xt": xt,
            "wqt": wqt, "wkt": wkt, "wvt": wvt, "wot": wot,
            "cosq": np.ascontiguousarray(cosT[:, tok] * inv).astype(bf16),
            "sinq": np.ascontiguousarray(sinT[:, tok] * inv).astype(bf16),
            "cosk": np.ascontiguousarray(cosT[:, tok]).astype(bf16),
            "sink": np.ascontiguousarray(sinT[:, tok]).astype(bf16),
            "mask1": mask1, "mask2": mask2,
            "bias": bias,
        })
    return in_maps


_NC_CACHE = None


def _get_nc():
    global _NC_CACHE
    if _NC_CACHE is None:
        _NC_CACHE = build_kernel()
    return _NC_CACHE


def kernel(x, wq, wk, wv, wo, freqs_cos, freqs_sin, _trace=False):
    x = np.asarray(x, dtype=np.float32)
    in_maps = _prep_inputs(
        x, np.asarray(wq, np.float32), np.asarray(wk, np.float32),
        np.asarray(wv, np.float32), np.asarray(wo, np.float32),
        np.asarray(freqs_cos, np.float32), np.asarray(freqs_sin, np.float32))
    nc = _get_nc()
    res = run_bass_kernel_spmd(nc, in_maps, core_ids=list(range(N_CORES)),
                               trace=_trace)
    out = np.empty((BSZ, SEQLEN, DIM), np.float32)
    for c in range(N_CORES):
        out[c // CPB, _tokens_of_core(c)] = res.results[c]["out"]
    if _trace:
        kernel.last_exec_time_ns = res.exec_time_ns
        kernel.last_results = res
    return out


if __name__ == "__main__":
    build_kernel()
    print("built ok")
